# revision 1
# baseline (speedup 1.0000x reference)
"""Trainium2 Bass kernel for nn_DialogueSNN (spiking net over vocab 32000).

Strategy
--------
Layer-1 (embedding lookup, fc1 [*,64]@[64,128], and the m1/spk1 leaky-
integrate-and-fire recurrence on [32,128]) is 0.1% of the FLOPs and is
computed on the host in fp32 with exactly the reference's elementwise op
order; its 0/1 spike train ships bit-packed (655KB).  The heavy work
runs on 8 NeuronCores, sharding the vocabulary (V=32000 padded to
32768, 4096 rows per core):

  - cur2 = spk1 @ W2.T on TensorE in float32r (TF32-like, 11-bit
    mantissa RNE) with a hi/lo 2-split of W2 -> ~22 mantissa bits at 2
    cycles/row, near-fp32 accuracy at half the cost of fp32 matmul.
    Output orientation [V_tile=128 partitions, steps*batch] so the
    membrane update runs on all 128 vector lanes.
  - The m2/spk2 recurrence (1280 sequential steps on [32, 4096] per
    core) runs on VectorE as ONE fused custom DVE op per step:
        m2 = (m2*beta + cur2) - (m2 > thr)
    bit-exact vs the reference's elementwise order (the reset r2 of
    step t equals spk2 of step t-1, so no separate spike tensor).
  - ScalarE drains PSUM->SBUF; GpSimd unpacks input bits and packs
    output spikes to bits; all wrapped in a For_i over the 64 tokens.

Only the final inner-step spike per token is emitted.
"""
import numpy as np

import concourse.bass as bass
import concourse.tile as tile
from concourse import bacc, mybir

# ---------------- problem constants (hardcoded per harness contract) -------
B, S, V, E, H = 32, 64, 32000, 64, 128
T = 20
BETA = np.float32(0.95)
THR = np.float32(1.0)
N_CORES = 8
VPAD = 32768
V_CORE = VPAD // N_CORES          # 4096 vocab rows per core
NTILE = V_CORE // 128             # 32 V-tiles of 128 per core
F = B * V_CORE // 128             # 1024 m2 elements per partition
NCHUNK = 2                        # chunks per token
CSTEP = T // NCHUNK               # 10 steps per chunk
NCOL = CSTEP * B                  # 320 rhs columns per chunk
TPS = 4                           # tiles per PSUM slot (512-aligned)
NSLOT = NTILE // TPS              # 8 slot fills per chunk
TB8 = T * B // 8                  # 80 packed spk1 bytes per token
F8 = F // 8                       # 128 packed output bytes per token

_DT = mybir.dt


# ---------------- custom DVE op: fused LIF step ----------------------------
def _register_lif_op():
    from concourse.dve_ops import DveOp, OPS, CUSTOM_DVE_SPECS, _SUB_OPCODE_FOR_NAME
    from concourse.dve_spec import Spec, Src0, Src1, C0, C1, lower
    from concourse.dve_uop import DveOpSpec

    name = "LIF_STEP_ANT"
    if name in _SUB_OPCODE_FOR_NAME:
        return next(op for op in OPS if op.name == name)
    body = ((Src0 * C0) + Src1) - (Src0 > C1)

    def ref(in0, in1, s0, s1, imm2):
        return (
            ((in0 * np.float32(s0)).astype(np.float32) + in1).astype(np.float32)
            - (in0 > np.float32(s1)).astype(np.float32)
        ).astype(np.float32)

    spec = Spec(body=body, reference=ref)
    row = max(_SUB_OPCODE_FOR_NAME.values()) + 1
    assert row < 0x20
    _SUB_OPCODE_FOR_NAME[name] = row
    shas = {}
    for ver in ("v3", "v4"):
        uops = lower(spec, ver=ver)
        shas[ver] = DveOpSpec(name=name, opcode=row, uops=uops, rd1_en=True).sha(ver)
    op = DveOp(name, spec, subdim=False, uops_sha=shas)
    OPS.append(op)
    CUSTOM_DVE_SPECS[name] = spec
    return op


# ---------------- host-side helpers ----------------------------------------
def _rne(x, bits):
    """Round fp32 array to `bits` explicit mantissa bits, nearest-even
    (bit-identical to the device float32r cast, HW-verified)."""
    u = x.view(np.uint32).astype(np.uint64)
    drop = 23 - bits
    half = np.uint64(1) << np.uint64(drop - 1)
    mask = (np.uint64(1) << np.uint64(drop)) - np.uint64(1)
    lsb = (u >> np.uint64(drop)) & np.uint64(1)
    rem = u & mask
    u2 = u >> np.uint64(drop)
    inc = (rem > half) | ((rem == half) & (lsb == 1))
    u2 = u2 + inc.astype(np.uint64)
    return ((u2 << np.uint64(drop)) & np.uint64(0xFFFFFFFF)).astype(np.uint32).view(
        np.float32
    )


def _spk1_host(x, embed, W1, b1):
    """Layer-1 spikes, fp32 elementwise exactly like the reference.
    Returns [S, T, B, H] float32 of 0/1."""
    emb = embed[x]                                            # [B, S, E]
    cur1 = (emb.reshape(-1, E).astype(np.float32) @ W1.T.astype(np.float32)).reshape(
        B, S, H
    ) + b1
    cur1 = cur1.astype(np.float32)
    m1 = np.zeros((B, H), np.float32)
    out = np.zeros((S, T, B, H), np.float32)
    for s in range(S):
        c = cur1[:, s, :]
        for t in range(T):
            r1 = (m1 > THR).astype(np.float32)
            m1 = ((BETA * m1 + c) - r1 * THR).astype(np.float32)
            out[s, t] = m1 - THR > 0
    return out


# ---------------- device module --------------------------------------------
TPB = 4 * TB8                      # packed bytes per token-pair lookahead DMA... (pair=2 tokens -> 2*TB8)


def _build(n_tokens=S, reps=1, variant="full"):
    assert n_tokens % 4 == 0
    lif_op = _register_lif_op()
    nc = bacc.Bacc("TRN2", target_bir_lowering=False, debug=False)

    npairs_pad = n_tokens // 2 + 2
    u8t = "-u8t" in variant
    spk1_d = nc.dram_tensor(
        "spk1b",
        [128, npairs_pad * 2 * (T * B if u8t else TB8)],
        _DT.uint8,
        kind="ExternalInput",
    ).ap()
    w2_d = nc.dram_tensor("w2t", [128, V_CORE], _DT.float32, kind="ExternalInput").ap()
    out_d = nc.dram_tensor(
        "spk_out", [128, n_tokens * F8], _DT.uint8, kind="ExternalOutput"
    ).ap()

    with tile.TileContext(nc) as tc:
        with tc.tile_pool(name="persist", bufs=1) as pp, tc.tile_pool(
            name="work", bufs=1
        ) as wp, tc.tile_pool(name="ps", bufs=2, space="PSUM") as psp, tc.tile_pool(
            name="ps2", bufs=4, space="PSUM"
        ) as psp2:
            # cur2 ring (3 buffers); w2f shares buffer 0 (prologue-only use)
            cur2 = [
                pp.tile([128, NCOL * NTILE], _DT.float32, tag=f"cur2_{b}",
                        name=f"cur2_{b}")
                for b in range(3)
            ]
            w2f = pp.tile([128, V_CORE], _DT.float32, tag="cur2_0", name="w2f")
            w2hi = pp.tile([128, V_CORE], _DT.float32r, tag="w2hi")
            w2lo = pp.tile([128, V_CORE], _DT.float32r, tag="w2lo")
            m2 = pp.tile([128, F], _DT.float32, tag="m2")
            nc.sync.dma_start(w2f[:], w2_d)
            # hi/lo float32r split of W2 (device cast == host _rne(.,11))
            nc.vector.tensor_copy(w2hi[:], w2f[:])
            nc.vector.tensor_tensor(
                w2f[:], w2f[:], w2hi[:].bitcast(_DT.float32), mybir.AluOpType.subtract
            )
            if "-blo" in variant:
                w2lo16 = pp.tile([128, V_CORE], _DT.bfloat16, tag="w2lo16")
                nc.vector.tensor_copy(w2lo16[:], w2f[:])
            else:
                w2lo16 = None
                nc.vector.tensor_copy(w2lo[:], w2f[:])
            nc.vector.memset(m2[:], 0.0)
            cnthr = pp.tile([128, 1], _DT.float32, tag="cnthr")
            nc.vector.memset(cnthr[:], -float(THR))

            # spk1 pair buffers (A: even pairs, B: odd pairs), f32r 0/1
            PW = 2 * T * B                          # 1280 cols per pair
            spk1A = pp.tile([128, PW], _DT.float32r, tag="spk1A")
            spk1B = pp.tile([128, PW], _DT.float32r, tag="spk1B")
            if "-blo" in variant:
                spk1A16 = pp.tile([128, PW], _DT.bfloat16, tag="spk1A16")
                spk1B16 = pp.tile([128, PW], _DT.bfloat16, tag="spk1B16")
                b16 = {id(spk1A): spk1A16, id(spk1B): spk1B16}
            spk_f = [
                pp.tile([128, F], _DT.float32, tag=f"spkf{b}", name=f"spkf{b}")
                for b in range(2)
            ]
            pk0 = pp.tile([128, F // 2], _DT.float32, tag="pk0")
            pk1 = pp.tile([128, F // 4], _DT.float32, tag="pk1")
            pk2 = pp.tile([128, F // 8], _DT.float32, tag="pk2")
            out_u8 = [
                pp.tile([128, F8], _DT.uint8, tag=f"outu8{b}", name=f"outu8{b}")
                for b in range(2)
            ]

            PB = 2 * TB8                            # 160 packed bytes per pair
            NBP = 8 if "-b16" in variant else 4     # pairs per loop body
            if NBP == 4:
                RING = [0, 1, 2, 0, 1, 2, 0, 1, 2, 0, 1, 2, 0, 1, 2, 1]
            else:
                RING = [u % 3 for u in range(30)] + [0, 1]

            def unpack_pair(dram_col_expr, buf, pre=""):
                """DMA one pair's packed spikes and expand into `buf` (f32r)."""
                PBX = (2 * T * B) if u8t else PB
                pck = wp.tile([128, PBX], _DT.uint8, tag=f"pck{pre}", name=f"pck{pre}")
                if dram_col_expr is None:
                    nc.sync.dma_start(pck[:], spk1_d[:, 0:PBX])
                else:
                    base, off = dram_col_expr
                    mult = (T * B) // TB8 if u8t else 1
                    nc.sync.dma_start(
                        pck[:],
                        spk1_d[:, off * mult:][:, bass.ds(base * mult, PBX)],
                    )
                if u8t:
                    nc.gpsimd.tensor_copy(buf[:], pck[:])
                    if "-blo" in variant:
                        nc.gpsimd.tensor_copy(b16[id(buf)][:], pck[:])
                    return
                ub = wp.tile([128, 8 * PB], _DT.uint8, tag=f"ub{pre}", name=f"ub{pre}")
                sview = buf[:].rearrange("p (j e) -> p j e", e=8)
                for k in range(8):
                    u = ub[:, k * PB:(k + 1) * PB]
                    nc.vector.tensor_scalar(
                        u, pck[:], k, 1,
                        mybir.AluOpType.logical_shift_right,
                        mybir.AluOpType.bitwise_and,
                    )
                    nc.gpsimd.tensor_copy(sview[:, :, k], u)

            def compute_token(buf, tok01, out_col, unit_base, phase):
                """Both chunks + LIF + spike emit for one token.

                buf: spk1 pair buffer; tok01: token within pair; out_col:
                (expr, offset) for the out DMA; unit_base: global unit index
                of this token's first chunk (selects cur2 ring buffers).
                """
                use2 = "-ps2" in variant
                sm = "-sm" in variant
                tps = 2 if use2 else TPS
                nslot = NTILE // tps
                for c in range(NCHUNK if not variant.startswith("nomm") else 0):
                    unit = unit_base + c
                    cc = cur2[RING[unit]]
                    rhs = buf[:, tok01 * (T * B) + c * NCOL:][:, 0:NCOL]
                    for sl in range(nslot if not variant.startswith("dveonly") else 0):
                        ps = (psp2 if use2 else psp).tile(
                            [128, tps * 512], _DT.float32, tag="ps"
                        )
                        for t4 in range(tps):
                            tt = sl * tps + t4
                            dst = ps[:, t4 * 512: t4 * 512 + NCOL]
                            nc.tensor.matmul(
                                dst, w2hi[:, tt * 128:(tt + 1) * 128], rhs,
                                start=True, stop=False,
                            )
                            if w2lo16 is not None:
                                rhs16 = b16[id(buf)][
                                    :, tok01 * (T * B) + c * NCOL:
                                ][:, 0:NCOL]
                                nc.tensor.matmul(
                                    dst, w2lo16[:, tt * 128:(tt + 1) * 128],
                                    rhs16, start=False, stop=True,
                                )
                            else:
                                nc.tensor.matmul(
                                    dst, w2lo[:, tt * 128:(tt + 1) * 128], rhs,
                                    start=False, stop=True,
                                )
                        if not variant.startswith("noact"):
                            ps_view = ps[:].rearrange("p (t x) -> p t x", t=tps)[
                                :, :, 0:NCOL
                            ]
                            if sm:
                                ccr = cc[:].rearrange(
                                    "p (t tt b) -> p tt t b", t=CSTEP, tt=NTILE
                                )
                                dst_sb = ccr[:, sl * tps:(sl + 1) * tps, :, :]
                            else:
                                dst_sb = cc[
                                    :, sl * (tps * NCOL):(sl + 1) * (tps * NCOL)
                                ]
                            nc.scalar.copy(dst_sb, ps_view)
                    cview = cc[:].rearrange(
                        "p (tt t b) -> p tt t b", tt=NTILE, t=CSTEP
                    )
                    for t in range(CSTEP if not (variant.startswith("nodve") or variant.startswith("noact")) else 0):
                        if "-l2" in variant:
                            H2 = NTILE // 2
                            for hh in range(2):
                                nc.vector._custom_dve(
                                    lif_op,
                                    out=m2[:, hh * (F // 2):(hh + 1) * (F // 2)],
                                    in0=m2[:, hh * (F // 2):(hh + 1) * (F // 2)],
                                    in1=cview[:, hh * H2:(hh + 1) * H2, t, :],
                                    s0=float(BETA), s1=float(THR),
                                )
                        else:
                            in1 = cc[:, t * F:(t + 1) * F] if sm else cview[:, :, t, :]
                            nc.vector._custom_dve(
                                lif_op, out=m2[:], in0=m2[:], in1=in1,
                                s0=float(BETA), s1=float(THR),
                            )
                # spikes of the last inner step -> bit-packed bytes
                sf = spk_f[phase]
                ou = out_u8[phase]
                if "-sg" in variant:
                    nc.scalar.activation(
                        sf[:], m2[:], mybir.ActivationFunctionType.Sign,
                        bias=cnthr[:], scale=1.0,
                    )
                    nc.scalar.activation(
                        sf[:], sf[:], mybir.ActivationFunctionType.Relu
                    )
                else:
                    nc.vector.tensor_scalar(
                        sf[:], m2[:], float(THR), None, mybir.AluOpType.is_gt
                    )
                if variant.startswith("nopack"):
                    nc.vector.tensor_copy(ou[:], sf[:, :F8])
                else:
                    sp2 = sf[:].rearrange("p (j e) -> p j e", e=2)
                    nc.gpsimd.tensor_tensor(
                        pk0[:], sp2[:, :, 1], sp2[:, :, 1], mybir.AluOpType.add
                    )
                    nc.gpsimd.tensor_tensor(
                        pk0[:], pk0[:], sp2[:, :, 0], mybir.AluOpType.add
                    )
                    p02 = pk0[:].rearrange("p (j e) -> p j e", e=2)
                    nc.gpsimd.tensor_tensor(
                        pk1[:], p02[:, :, 1], p02[:, :, 1], mybir.AluOpType.add
                    )
                    nc.gpsimd.tensor_tensor(
                        pk1[:], pk1[:], pk1[:], mybir.AluOpType.add
                    )
                    nc.gpsimd.tensor_tensor(
                        pk1[:], pk1[:], p02[:, :, 0], mybir.AluOpType.add
                    )
                    p12 = pk1[:].rearrange("p (j e) -> p j e", e=2)
                    nc.gpsimd.tensor_tensor(
                        pk2[:], p12[:, :, 1], p12[:, :, 1], mybir.AluOpType.add
                    )
                    nc.gpsimd.tensor_tensor(
                        pk2[:], pk2[:], pk2[:], mybir.AluOpType.add
                    )
                    nc.gpsimd.tensor_tensor(
                        pk2[:], pk2[:], pk2[:], mybir.AluOpType.add
                    )
                    nc.gpsimd.tensor_tensor(
                        pk2[:], pk2[:], pk2[:], mybir.AluOpType.add
                    )
                    nc.gpsimd.tensor_tensor(
                        pk2[:], pk2[:], p12[:, :, 0], mybir.AluOpType.add
                    )
                    nc.gpsimd.tensor_copy(ou[:], pk2[:])
                base, off = out_col
                nc.sync.dma_start(out_d[:, off:][:, bass.ds(base, F8)], ou[:])

            def body(j):
                # iteration j handles NBP pairs (2*NBP tokens), alternating
                # buffers A/B with one-pair unpack lookahead.
                jb = j * (2 * NBP * F8)
                jp = j * (NBP * PB)
                for k in range(NBP):
                    buf = spk1A if k % 2 == 0 else spk1B
                    nbuf = spk1B if k % 2 == 0 else spk1A
                    unpack_pair((jp, (k + 1) * PB), nbuf, pre="ab"[k % 2])
                    compute_token(buf, 0, (jb, (2 * k) * F8), 4 * k, 0)
                    compute_token(buf, 1, (jb, (2 * k + 1) * F8), 4 * k + 2, 1)

            # prologue: unpack pair 0 -> A
            unpack_pair(None, spk1A, pre="p")

            assert n_tokens % (2 * NBP) == 0
            nit = n_tokens // (2 * NBP)
            if variant.endswith("-unroll"):
                class _FakeReg(int):
                    pass
                for jj in range(nit):
                    body(jj)
                nit = 0
            stag = variant.endswith("-stag")
            hints = ()
            if variant.endswith("-hint"):
                hints = (
                    mybir.EngineType.PE,
                    mybir.EngineType.Activation,
                    mybir.EngineType.DVE,
                    mybir.EngineType.Pool,
                )
            if nit == 0:
                pass
            elif reps == 1:
                with tc.For_i(0, nit, 1, staggered_reset=stag,
                              hint_engines=hints) as j:
                    body(j)
            else:
                with tc.For_i(0, reps, 1) as _r:
                    with tc.For_i(0, nit, 1, staggered_reset=stag,
                                  hint_engines=hints) as j:
                        body(j)

    nc.finalize()
    return nc


# ---------------- cached PJRT runner ----------------------------------------
_NC_CACHE = {}
_RUN_CACHE = {}


def _get_nc(key):
    if key not in _NC_CACHE:
        _NC_CACHE[key] = _build(*key)
    return _NC_CACHE[key]


def _get_runner(key):
    """Build (once) a cached jitted SPMD executor for the module."""
    if key in _RUN_CACHE:
        return _RUN_CACHE[key]
    import jax
    from jax.sharding import Mesh, PartitionSpec
    from jax.experimental.shard_map import shard_map
    from concourse import bass2jax
    from concourse.bass2jax import (
        _bass_exec_p, install_neuronx_cc_hook, partition_id_tensor,
    )

    install_neuronx_cc_hook()
    nc = _get_nc(key)
    assert nc.dbg_addr is None
    pid_name = nc.partition_id_tensor.name if nc.partition_id_tensor else None

    in_names, out_names, out_avals = [], [], []
    for alloc in nc.m.functions[0].allocations:
        if not isinstance(alloc, mybir.MemoryLocationSet):
            continue
        name = alloc.memorylocations[0].name
        if alloc.kind == "ExternalInput":
            if name == pid_name:
                continue
            in_names.append(name)
        elif alloc.kind == "ExternalOutput":
            out_names.append(name)
            out_avals.append(
                jax.core.ShapedArray(tuple(alloc.tensor_shape), mybir.dt.np(alloc.dtype))
            )
    n_params = len(in_names)
    all_names = tuple(in_names + out_names) + ((pid_name,) if pid_name else ())

    def _body(*args):
        operands = list(args)
        if pid_name:
            operands.append(partition_id_tensor())
        outs = _bass_exec_p.bind(
            *operands,
            out_avals=tuple(out_avals),
            in_names=all_names,
            out_names=tuple(out_names),
            lowering_input_output_aliases=(),
            sim_require_finite=True,
            sim_require_nnan=True,
            nc=nc,
        )
        return tuple(outs)

    devices = jax.devices()[:N_CORES]
    assert len(devices) >= N_CORES, f"need {N_CORES} devices, have {len(devices)}"
    mesh = Mesh(np.asarray(devices), ("core",))
    n_outs = len(out_names)
    sharded = jax.jit(
        shard_map(
            _body,
            mesh=mesh,
            in_specs=(PartitionSpec("core"),) * (n_params + n_outs),
            out_specs=(PartitionSpec("core"),) * n_outs,
            check_rep=False,
        ),
        donate_argnums=tuple(range(n_params, n_params + n_outs)),
        keep_unused=True,
    )
    runner = (sharded, in_names, out_names, out_avals)
    _RUN_CACHE[key] = runner
    return runner


def _run_spmd(key, in_maps):
    sharded, in_names, out_names, out_avals = _get_runner(key)
    concat_in = [
        np.concatenate([in_maps[c][n] for c in range(N_CORES)], axis=0)
        for n in in_names
    ]
    zeros = [
        np.zeros((N_CORES * a.shape[0], *a.shape[1:]), a.dtype) for a in out_avals
    ]
    out_arrs = sharded(*concat_in, *zeros)
    return [
        {
            n: np.asarray(out_arrs[j]).reshape(N_CORES, *out_avals[j].shape)[c]
            for j, n in enumerate(out_names)
        }
        for c in range(N_CORES)
    ]


# ---------------- public entry point ----------------------------------------
def kernel(x, embed, W1, b1, W2, b2, _n_tokens=S, _reps=1, _return_raw=False):
    x = np.asarray(x)
    embed = np.asarray(embed, np.float32)
    W1 = np.asarray(W1, np.float32)
    b1 = np.asarray(b1, np.float32)
    W2 = np.asarray(W2, np.float32)
    b2 = np.asarray(b2, np.float32)

    # host: layer-1 spikes -> uint8 rhs [128, S*T*B] (+lookahead pad)
    spk1 = _spk1_host(x, embed, W1, b1)                    # [S, T, B, H]
    spk1_rhs = np.ascontiguousarray(spk1.reshape(S * T * B, H).T)
    spk1_bits = np.concatenate(
        [spk1_rhs.astype(np.uint8), np.zeros((128, 4 * T * B), np.uint8)], axis=1
    )

    # host: W2 pad + transpose; hi/lo split happens on device
    W2p = np.zeros((VPAD, H), np.float32)
    W2p[:V] = W2
    W2Tp = np.ascontiguousarray(W2p.T)                     # [128, VPAD]

    in_maps = []
    for k in range(N_CORES):
        sl = slice(k * V_CORE, (k + 1) * V_CORE)
        in_maps.append(
            {"spk1b": spk1_bits, "w2t": np.ascontiguousarray(W2Tp[:, sl])}
        )

    key = (_n_tokens, _reps, "full-u8t")
    results = _run_spmd(key, in_maps)
    if _return_raw:
        return results

    out = np.empty((B, S, VPAD), np.float32)
    for k in range(N_CORES):
        bits = np.unpackbits(
            results[k]["spk_out"], axis=1, bitorder="little"
        )                                                   # [128, S*F]
        o = bits.reshape(128, S, NTILE, B)                  # [p, s, tau, b]
        out[:, :, k * V_CORE:(k + 1) * V_CORE] = o.transpose(3, 1, 2, 0).reshape(
            B, S, V_CORE
        )
    return np.ascontiguousarray(out[:, :, :V])

